# revision 43
# baseline (speedup 1.0000x reference)
"""Trainium2 Bass kernel for nn_DeletionChannel.

Strategy
--------
Pure data parallelism: batch B=128 is sharded 16 rows per core across 8
NeuronCores. Inside a core, the 16 batch rows are laid out as 2 "halves"
of 8 rows each; the partition dim is (blk in 0..8) x (l in 0..10) = 80
partitions, and the two halves ride side by side in the free dim. All
cross-`l` mixing becomes block-diagonal constant matmuls on the tensor
engine; per-(b,l) softmax norms are per-partition scalars.

Math simplifications vs the reference:
 * The [B, 2^L, L, V] combo logsumexp collapses to a 10x10 row-stochastic
   matrix A applied in linear space: A[m,l] = sum_{c: perm[c,m]=l} exp(scl[m,c]).
 * The sequential EOS renormalization has the closed form
   col_j = ln(p_j) - ln(1 - sum_{j'<j} p_{j'}) with p the adjusted length pmf.
 * The deletion shift is a per-batch 0/1 permutation matrix built from an
   exclusive cumsum of the keep mask and an equality compare, applied as a
   bf16 matmul (all operands 0/1 -> exact); the EOS tail fill is a rank-1
   correction folded into the same PSUM accumulation group.

Performance notes (raw Bacc, manual semaphores):
 * No Tile context: Tile's kernel-tail drain/barrier machinery costs ~9us.
 * One activation-table load: the act-table pass is restricted to the
   combined natural_log_exp_and_others set so Exp/Ln never thrash the
   table SRAM (1.28us per reload otherwise).
 * The dead const-AP memsets + init all-engine barrier from Bass.__init__
   are stripped; they would otherwise start the measured window early.
 * Input and output DMAs are split across the two HWDGE rings (SP + ACT)
   so transfers and their ~2us completion latencies overlap.
"""

import numpy as np
import itertools
import math
import ml_dtypes

from concourse import bacc, bass, mybir
from concourse.bass_utils import run_bass_kernel_spmd
from concourse.mybir import ActivationFunctionType as AF, AluOpType as ALU

# The act-table insertion pass greedily picks the first table set containing
# each activation function, so alternating Exp/Ln thrashes the table SRAM
# (1.28us reload per switch). Restrict the choice to the one combined set
# (natural_log_exp_and_others: Exp+Ln+Copy+Identity) so a single load at
# kernel start covers every activation in this kernel.
_orig_get_act_tables = bacc.get_activation_tables


def _combined_act_tables(arch):
    t = _orig_get_act_tables(arch)
    return {name: (funcs if name == "natural_log_exp_and_others" else set())
            for name, funcs in t.items()}


bacc.get_activation_tables = _combined_act_tables

P_ERR = 0.1
B, L, V = 128, 10, 32
NCORES = 8
BS = B // NCORES            # batch rows per core = 16
NB = 8                      # blocks per half
NH = 2                      # halves per core
P80 = NB * L                # 80 partitions, (blk, l)
P88 = NB * (L + 1)          # 88 partitions, (blk, j)
MIN = float(np.finfo(np.float32).min)
F32 = mybir.dt.float32
BF16 = mybir.dt.bfloat16
BF = ml_dtypes.bfloat16

# bundle column layout (f32 columns; bf16 data packed 2-per-column).
# Region 1 (first DMA): inputs + everything on the critical dependency path.
HVW = V + 1                 # per-half: log(32) | mask(1)
B_HV = 0                    # [66]  h-major log/mask
B_ONE = B_HV + NH * HVW     # [1]   ones column (activation bias)
B_ZERO = B_ONE + 1          # [1]   zeros column (activation bias)
B_MSGB = B_ZERO + 1         # [32]  bf16 messages, h-major 16 cols each
B_E2 = B_MSGB + V           # [88]  block-diag identity injection 80->88
B_T = B_E2 + P88            # [88]  block-diag T (i<j), j in 0..10
B_UEXB = B_T + P88          # [40]  bf16 block-diag strict-lower cumsum
B_W0 = B_UEXB + P80 // 2    # start of the weights region (second DMA)
B_BDA = B_W0                # [80]  block-diag A^T (lhsT for expected)
B_IOTA = B_BDA + P80        # [80]  row iota 0..79
B_BLK = B_IOTA + P80        # [1]   10*blk per partition
B_ONESB = B_BLK + 1         # [40]  bf16 partition-0 ones row
B_E0B = B_ONESB + P80 // 2  # [32]  bf16 partition-0 e0-per-half row
NBUND = B_E0B + V
# const88 column layout
C_P10 = 0                   # [80]  block-diag NDLe[j, j2]
C_S10 = C_P10 + P80         # [80]  block-diag exclusive-cumsum NDLe
C_SP10 = C_S10 + P80        # [80]  P10 + S10
C_ZERO88 = C_SP10 + P80     # [1]   zeros (activation bias)
NC88 = C_ZERO88 + 1


def _host_constants():
    """A [10,10] row-stochastic mix matrix and NDLe [11,11] binomial pmf."""
    combos = np.array(list(itertools.product((0, 1), repeat=L)), dtype=bool)
    n_del = combos.sum(-1)
    combo_logits = np.log(P_ERR) * n_del + np.log1p(-P_ERR) * (L - n_del)
    not_del = np.arange(L - 1, -1, -1)[:, None] >= n_del[None, :]
    scl = np.where(not_del, combo_logits[None, :], MIN)
    m = scl.max(-1, keepdims=True)
    scl = scl - (m + np.log(np.exp(scl - m).sum(-1, keepdims=True)))  # [L, C]
    perm = np.tile(np.arange(L), (len(combos), 1))
    for i in range(1, L):
        idx = L - 1 - i
        t = combos[:, idx]
        perm[t, idx:] = np.roll(perm[t, idx:], -1, axis=1)
    A = np.zeros((L, L))
    for l in range(L):
        for lp in range(L):
            sel = scl[l, perm[:, l] == lp]
            if len(sel):
                mm = sel.max()
                if mm > MIN / 2:
                    A[l, lp] = np.exp(sel - mm).sum() * np.exp(mm)
    ndl = np.full((L + 1, L + 1), MIN)
    for n in range(L + 1):
        for k in range(n + 1):
            ndl[n, n - k] = (math.lgamma(n + 1) - math.lgamma(k + 1)
                             - math.lgamma(n - k + 1)
                             + k * math.log(P_ERR) + (n - k) * math.log(1 - P_ERR))
    NDLe = np.exp(np.where(ndl <= MIN / 2, -np.inf, ndl))
    return A, NDLe


def _pack_bf16(x):
    """Pack a [..., 2k] float array as bf16 pairs into [..., k] f32 columns."""
    xb = np.ascontiguousarray(x.astype(BF))
    assert xb.shape[-1] % 2 == 0
    return xb.view(np.uint16).view(np.uint32).view(np.float32)


def _const_blobs():
    """Constant parts of the bundle ([80, NBUND] template) and c88."""
    A, NDLe = _host_constants()
    c80 = np.zeros((P80, NBUND), np.float32)
    c88 = np.zeros((P88, NC88), np.float64)
    uex = np.zeros((P80, P80), np.float32)
    for blk in range(NB):
        r0, r1 = blk * L, (blk + 1) * L          # 80-layout rows of this block
        q0 = blk * (L + 1)                        # 88-layout base
        # BD_A[(blk,l'), (blk,l)] = A[l, l']
        c80[r0:r1, B_BDA + r0:B_BDA + r1] = A.T
        uex[r0:r1, r0:r1] = np.triu(np.ones((L, L)), k=1)
        Tm = np.zeros((L, L + 1))
        for i in range(L):
            Tm[i, i + 1:] = 1.0
        c80[r0:r1, B_T + q0:B_T + q0 + L + 1] = Tm
        E2 = np.zeros((L, L + 1))
        E2[:, :L] = np.eye(L)
        c80[r0:r1, B_E2 + q0:B_E2 + q0 + L + 1] = E2
        P10 = NDLe[:, :L]
        S10 = np.cumsum(NDLe, axis=1)[:, :L] - P10
        c88[q0:q0 + L + 1, C_P10 + r0:C_P10 + r1] = P10
        c88[q0:q0 + L + 1, C_S10 + r0:C_S10 + r1] = S10
        c88[q0:q0 + L + 1, C_SP10 + r0:C_SP10 + r1] = P10 + S10
    c80[:, B_IOTA:B_IOTA + P80] = np.arange(P80)[None, :]
    c80[:, B_BLK] = (np.arange(P80) // L) * L
    c80[:, B_ONE] = 1.0
    c80[:, B_UEXB:B_UEXB + P80 // 2] = _pack_bf16(uex)
    ones_row = np.zeros((1, P80), np.float32)
    ones_row[0, :] = 1.0
    c80[0:1, B_ONESB:B_ONESB + P80 // 2] = _pack_bf16(ones_row)
    e0 = np.zeros((1, NH * V), np.float32)
    e0[0, 0] = 1.0
    e0[0, V] = 1.0
    c80[0:1, B_E0B:B_E0B + V] = _pack_bf16(e0)
    return c80, c88.astype(np.float32)


def _strip_init_overhead(nc):
    """Remove the dead const-AP memsets and the init all-engine barrier that
    Bass.__init__ emits; nothing in this kernel reads the const APs, and all
    cross-engine ordering is established by this kernel's own semaphores."""
    b = nc.main_func.blocks[0]
    drop = [i for i in b.instructions
            if type(i).__name__ in ("InstMemset", "InstDrain",
                                    "InstEventSemaphore")]
    for i in drop:
        b.instructions.remove(i)


def build_program():
    """Raw Bacc program: manual semaphores, no Tile machinery.

    Engine streams (each internally ordered; cross-engine deps via sems):
      SP:   dma-in hv+msgb, dma-in c88, dma-out noisy, completion wait
      ACT:  dma-in weights, memzero, exp_t x2, exp_eos, log1m, p_len, logE,
            ln_p, ln_q, ln_qm, noisy copy, dma-out adjusted   (sA +1 each)
      PE:   E2, dest, T, E, P, S, SP, G0, G1, ones-fix        (sP +1 each)
      DVE:  keep, keepb, msg0-=1, rs, p_ne x2, sdest, G x2, eos_d, l1me,
            adj0, cvec, rest x2                               (sV +1 each)
    """
    nc = bacc.Bacc("TRN2", target_bir_lowering=False, debug=False)
    _strip_init_overhead(nc)
    d_bund = nc.dram_tensor("bundle", [P80, NBUND], F32, kind="ExternalInput")
    d_c88 = nc.dram_tensor("const88", [P88, NC88], F32, kind="ExternalInput")
    # outputs are partition-major [80, 2*V]; the host reassembles batch order
    d_noisy = nc.dram_tensor("noisy", [P80, NH * V], F32, kind="ExternalOutput")
    d_adj = nc.dram_tensor("adjusted", [P80, NH * V], F32, kind="ExternalOutput")

    sDb = nc.alloc_semaphore("sDb")   # hv + msgb region
    sDw = nc.alloc_semaphore("sDw")   # weights region
    sDc = nc.alloc_semaphore("sDc")   # c88
    sP = nc.alloc_semaphore("sP")
    sA = nc.alloc_semaphore("sA")
    sV = nc.alloc_semaphore("sV")
    sO = nc.alloc_semaphore("sO")

    bund = nc.alloc_sbuf_tensor("bund", [P80, NBUND], F32)
    c88 = nc.alloc_sbuf_tensor("c88", [P88, NC88], F32)
    keep = nc.alloc_sbuf_tensor("keep", [P80, NH], F32)
    keepb = nc.alloc_sbuf_tensor("keepb", [P80, NH], BF16)
    sdest = nc.alloc_sbuf_tensor("sdest", [P80, NH], F32)
    G = nc.alloc_sbuf_tensor("G", [P80, NH, P80], BF16)
    noisy_sb = nc.alloc_sbuf_tensor("noisy_sb", [P80, NH, V], F32)
    exp_t = nc.alloc_sbuf_tensor("exp_t", [P80, NH, V - 1], F32)
    s_t = nc.alloc_sbuf_tensor("s_t", [P80, NH], F32)
    rs = nc.alloc_sbuf_tensor("rs", [P80, NH], F32)
    rhsE = nc.alloc_sbuf_tensor("rhsE", [P80, NH, V - 1], F32)
    logE = nc.alloc_sbuf_tensor("logE", [P80, NH, V - 1], F32)
    exp_eos = nc.alloc_sbuf_tensor("exp_eos", [P80, NH], F32)
    log1m = nc.alloc_sbuf_tensor("log1m", [P80, NH], F32)
    p_len = nc.alloc_sbuf_tensor("p_len", [P88, NH], F32)
    ln_p = nc.alloc_sbuf_tensor("ln_p", [P80, NH], F32)
    ln_q = nc.alloc_sbuf_tensor("ln_q", [P80, NH], F32)
    ln_qm = nc.alloc_sbuf_tensor("ln_qm", [P80, NH], F32)
    l1me = nc.alloc_sbuf_tensor("l1me", [P80, NH], F32)
    adj_out = nc.alloc_sbuf_tensor("adj_out", [P80, NH, V], F32)

    s_sb = nc.alloc_sbuf_tensor("s_sb", [P80, NH], F32)
    p_sb = nc.alloc_sbuf_tensor("p_sb", [P80, NH], F32)
    dest_ps = nc.alloc_psum_tensor("dest_ps", [P80, NH], F32)
    noisy_ps = nc.alloc_psum_tensor("noisy_ps", [P80, NH, V], F32)
    E_ps = nc.alloc_psum_tensor("E_ps", [P80, NH, V - 1], F32)
    LL_ps = nc.alloc_psum_tensor("LL_ps", [P88, NH], F32)
    S_ps = nc.alloc_psum_tensor("S_ps", [P80, NH], F32)
    SP_ps = nc.alloc_psum_tensor("SP_ps", [P80, NH], F32)

    hv = bund[:, B_HV:B_HV + NH * HVW].rearrange("p (h x) -> p h x", h=NH)
    log_t = hv[:, :, 0:V]
    mask_t = hv[:, :, V:V + 1]
    msgb = bund[:, B_MSGB:B_MSGB + V].bitcast(BF16).rearrange(
        "p (h x) -> p h x", h=NH)          # [80, 2, 32] bf16
    ones80 = bund[:, B_ONE:B_ONE + 1]
    zero80 = bund[:, B_ZERO:B_ZERO + 1]
    zero88 = c88[:, C_ZERO88:C_ZERO88 + 1]
    uexb = bund[:, B_UEXB:B_UEXB + P80 // 2].bitcast(BF16)    # [80, 80]
    onesb = bund[0:1, B_ONESB:B_ONESB + P80 // 2].bitcast(BF16)  # [1, 80]
    e0b = bund[0:1, B_E0B:B_E0B + V].bitcast(BF16)            # [1, 64]

    # ---- SP: c88 input DMA ----
    nc.sync.dma_start(out=c88[:, :], in_=d_c88[:, :]).then_inc(sDc, 16)

    # ---- ACT stream (triggers both bundle DMAs on the ACT ring) ----
    # ACT (and DVE) pipelines do not interlock same-engine RAW: a consumer
    # must wait on the producer's sem tick even on the same engine.
    # The first activation (exp_eos) carries the DMA wait itself, so the act
    # table load the compiler hoists before it runs during the input DMAs,
    # and the measured window starts only when real compute starts.
    nc.scalar.dma_start(
        out=bund[:, 0:B_W0], in_=d_bund[:, 0:B_W0]).then_inc(sDb, 16)
    nc.scalar.dma_start(
        out=bund[:, B_W0:NBUND], in_=d_bund[:, B_W0:NBUND]).then_inc(sDw, 16)
    # DVE-stream tick of the p = SP - S diff, needed by ACT's ln_p wait
    # before the DVE stream is emitted (asserted at the definition site).
    V_PDIFF = 12
    a = 0
    nc.scalar.wait_ge(sDb, 16)
    # EOS/length chain first: it feeds the longest dependency chain.
    nc.scalar.activation(exp_eos[:, :], log_t[:, :, 0], AF.Exp,
                         bias=zero80, scale=1.0).then_inc(sA, 1)
    a += 1
    nc.scalar.wait_ge(sA, a)           # same-engine RAW on exp_eos
    nc.scalar.activation(log1m[:, :], exp_eos[:, :], AF.Ln,
                         bias=ones80, scale=-1.0).then_inc(sA, 1)
    a += 1
    A_LOG1M = a
    for h in range(NH):
        nc.scalar.activation(exp_t[:, h, :], log_t[:, h, 1:V], AF.Exp,
                             bias=zero80, scale=1.0).then_inc(sA, 1)
        a += 1
    A_EXPT = a
    nc.scalar.wait_ge(sDc, 16)         # zero88 bias below lives in c88
    nc.scalar.wait_ge(sP, 3)           # LL accumulation done
    nc.scalar.activation(p_len[:, :], LL_ps[:, :], AF.Exp,
                         bias=zero88, scale=1.0).then_inc(sA, 1)
    a += 1
    A_PLEN = a
    nc.scalar.wait_ge(sP, 4)           # S done
    nc.scalar.activation(ln_q[:, :], S_ps[:, :], AF.Ln,
                         bias=ones80, scale=-1.0).then_inc(sA, 1)
    a += 1
    A_LNQ = a
    nc.scalar.wait_ge(sP, 5)           # SP done
    nc.scalar.activation(ln_qm[:, :], SP_ps[:, :], AF.Ln,
                         bias=ones80, scale=-1.0).then_inc(sA, 1)
    a += 1
    A_LNQM = a
    nc.scalar.wait_ge(sV, V_PDIFF)     # p = SP - S computed on DVE
    nc.scalar.activation(ln_p[:, :], p_sb[:, :], AF.Ln,
                         bias=zero80, scale=1.0).then_inc(sA, 1)
    a += 1
    A_LNP = a
    nc.scalar.wait_ge(sP, 6)           # E matmul done
    nc.scalar.activation(logE[:, :, :], E_ps[:, :, :], AF.Ln,
                         bias=zero80, scale=1.0).then_inc(sA, 1)
    a += 1
    A_LOGE = a
    nc.scalar.wait_ge(sP, 9)           # noisy matmuls done
    nc.scalar.copy(noisy_sb[:, :, :], noisy_ps[:, :, :]).then_inc(sA, 1)
    a += 1
    A_NCOPY = a

    # ---- DVE stream ----
    v = 0
    nc.vector.wait_ge(sDb, 16)
    nc.vector.tensor_scalar(
        keep[:, :], mask_t[:, :, 0], -1.0, 1.0, ALU.mult, ALU.add).then_inc(sV, 1)
    v += 1
    nc.vector.tensor_scalar(
        keepb[:, :], mask_t[:, :, 0], -1.0, 1.0, ALU.mult, ALU.add).then_inc(sV, 1)
    v += 1
    V_KEEPB = v
    # messages col0 -= 1 so the rank-1 ones fix yields the EOS tail
    nc.vector.tensor_scalar(
        msgb[:, :, 0], msgb[:, :, 0], -1.0, None, ALU.add).then_inc(sV, 1)
    v += 1
    nc.vector.wait_ge(sA, A_EXPT)
    nc.vector.tensor_reduce(
        s_t[:, :], exp_t[:, :, :], mybir.AxisListType.X,
        ALU.add).then_inc(sV, 1)
    v += 1
    nc.vector.wait_ge(sV, v)           # same-engine RAW on s_t
    nc.vector.reciprocal(rs[:, :], s_t[:, :]).then_inc(sV, 1)
    v += 1
    nc.vector.wait_ge(sV, v)           # same-engine RAW on rs
    for h in range(NH):
        nc.vector.tensor_scalar(
            rhsE[:, h, :], exp_t[:, h, :], rs[:, h:h + 1], None,
            ALU.mult).then_inc(sV, 1)
        v += 1
    V_PNE = v
    nc.vector.wait_ge(sDw, 16)         # IOTA/BLK columns live in weights rgn
    nc.vector.wait_ge(sP, 2)           # dest matmul done
    nc.vector.tensor_scalar(
        sdest[:, :], dest_ps[:, :], bund[:, B_BLK:B_BLK + 1], None,
        ALU.add).then_inc(sV, 1)
    v += 1
    nc.vector.wait_ge(sV, v)           # same-engine RAW on sdest
    for h in range(NH):
        nc.vector.tensor_scalar(
            G[:, h, :], bund[:, B_IOTA:B_IOTA + P80],
            sdest[:, h:h + 1], keep[:, h:h + 1],
            ALU.is_equal, ALU.mult).then_inc(sV, 1)
        v += 1
    V_G = v
    # p = (S+P) - S from the two PSUM accumulations (dropping the third
    # matmul); ln_p deviation vs the direct product is ~8e-6 on this data
    nc.vector.wait_ge(sP, 4)           # S matmul done
    nc.vector.tensor_copy(s_sb[:, :], S_ps[:, :]).then_inc(sV, 1)
    v += 1
    nc.vector.wait_ge(sP, 5)           # SP matmul done
    nc.vector.wait_ge(sV, v)           # same-engine RAW on s_sb
    nc.vector.tensor_tensor(
        p_sb[:, :], SP_ps[:, :], s_sb[:, :], ALU.subtract).then_inc(sV, 1)
    v += 1
    assert v == V_PDIFF
    # adjusted col 0 = ln_p - ln_q written in place (the reference's clip to
    # <=0 only guards f32 rounding at the ~1e-7 level; p < q by construction)
    nc.vector.wait_ge(sA, A_LNP)
    nc.vector.tensor_tensor(
        adj_out[:, :, 0], ln_p[:, :], ln_q[:, :], ALU.subtract).then_inc(sV, 1)
    v += 1
    nc.vector.wait_ge(sA, A_LNQM)
    nc.vector.tensor_tensor(
        l1me[:, :], ln_qm[:, :], ln_q[:, :], ALU.subtract).then_inc(sV, 1)
    v += 1
    V_L1ME = v
    nc.vector.wait_ge(sA, A_LOGE)
    nc.vector.wait_ge(sV, V_L1ME)      # same-engine RAW on l1me
    for h in range(NH):
        nc.vector.tensor_scalar(
            adj_out[:, h, 1:V], logE[:, h, :],
            l1me[:, h:h + 1], None, ALU.add).then_inc(sV, 1)
        v += 1
    V_ADJ = v

    # ---- PE stream ----
    p = 0
    nc.tensor.wait_ge(sDb, 16)
    nc.tensor.matmul(LL_ps[:, :], bund[:, B_E2:B_E2 + P88], log_t[:, :, 0],
                     start=True, stop=False).then_inc(sP, 1)
    p += 1
    nc.tensor.wait_ge(sV, V_KEEPB)
    nc.tensor.matmul(dest_ps[:, :], uexb, keepb[:, :]).then_inc(sP, 1)
    p += 1
    nc.tensor.wait_ge(sA, A_LOG1M)
    nc.tensor.matmul(LL_ps[:, :], bund[:, B_T:B_T + P88], log1m[:, :],
                     start=False, stop=True).then_inc(sP, 1)
    p += 1
    nc.tensor.wait_ge(sDc, 16)
    nc.tensor.wait_ge(sA, A_PLEN)
    nc.tensor.matmul(S_ps[:, :], c88[:, C_S10:C_S10 + P80],
                     p_len[:, :]).then_inc(sP, 1)
    p += 1
    nc.tensor.matmul(SP_ps[:, :], c88[:, C_SP10:C_SP10 + P80],
                     p_len[:, :]).then_inc(sP, 1)
    p += 1
    nc.tensor.wait_ge(sDw, 16)
    nc.tensor.wait_ge(sV, V_PNE)
    nc.tensor.matmul(E_ps.ap().rearrange("p a b -> p (a b)"),
                     bund[:, B_BDA:B_BDA + P80],
                     rhsE.ap().rearrange("p a b -> p (a b)")).then_inc(sP, 1)
    p += 1
    nc.tensor.wait_ge(sV, V_G)
    for h in range(NH):
        # start only on h=0: start zeroes the whole PSUM bank (2KB zero
        # region), so h=1 must accumulate into the already-zeroed half.
        nc.tensor.matmul(noisy_ps[:, h, :], G[:, h, :], msgb[:, h, :],
                         start=(h == 0), stop=False,
                         skip_group_check=True).then_inc(sP, 1)
        p += 1
    nc.tensor.matmul(noisy_ps.ap().rearrange("p a b -> p (a b)"),
                     onesb, e0b, start=False, stop=True,
                     skip_group_check=True).then_inc(sP, 1)
    p += 1

    # ---- output DMAs: noisy on the SP ring, adjusted on the ACT ring.
    # No final completion wait: the NEFF epilogue drains each triggering
    # engine's DMA rings before the semaphore resets. (A 4-way row-split
    # across rings was measured slower: +1.6us from the extra triggers.)
    nc.sync.wait_ge(sA, A_NCOPY)
    nc.sync.dma_start(
        out=d_noisy[:, :].rearrange("p (h v) -> p h v", h=NH),
        in_=noisy_sb[:, :, :]).then_inc(sO, 16)
    nc.scalar.wait_ge(sV, V_ADJ)
    nc.scalar.dma_start(
        out=d_adj[:, :].rearrange("p (h v) -> p h v", h=NH),
        in_=adj_out[:, :, :]).then_inc(sO, 16)

    nc.compile()
    return nc


_PROGRAM = None
_CONSTS = None


def _get_program():
    global _PROGRAM, _CONSTS
    if _PROGRAM is None:
        _PROGRAM = build_program()
        _CONSTS = _const_blobs()
    return _PROGRAM, _CONSTS


def _bundles(messages, logits, maskf, c80):
    """Per-core [80, NBUND] bundles: log/mask halves + bf16 msg + constants."""
    msg2 = messages.reshape(B * L, V)
    log2 = logits.reshape(B * L, V)
    mask2 = maskf.reshape(B * L)
    out = []
    for c in range(NCORES):
        base = c * BS * L
        bund = c80.copy()
        for h in range(NH):
            r = slice(base + h * P80, base + (h + 1) * P80)
            o = B_HV + h * HVW
            bund[:, o:o + V] = log2[r]
            bund[:, o + V] = mask2[r]
            bund[:, B_MSGB + h * (V // 2):B_MSGB + (h + 1) * (V // 2)] = (
                _pack_bf16(msg2[r]))
        out.append(bund)
    return out


def _run(messages, logits, target_mask, **spmd_kwargs):
    messages = np.ascontiguousarray(np.asarray(messages, np.float32))
    logits = np.ascontiguousarray(np.asarray(logits, np.float32))
    maskf = np.ascontiguousarray(np.asarray(target_mask).astype(np.float32))
    nc, (c80, c88) = _get_program()
    in_maps = [{"bundle": b, "const88": c88}
               for b in _bundles(messages, logits, maskf, c80)]
    res = run_bass_kernel_spmd(
        nc, in_maps, core_ids=list(range(NCORES)), **spmd_kwargs)

    def unshard(name):
        # [80, 2*V] partition-major -> batch-major [16, 10, 32] per core
        parts = []
        for c in range(NCORES):
            a = res.results[c][name].reshape(P80, NH, V)
            parts.append(np.ascontiguousarray(
                a.transpose(1, 0, 2)).reshape(BS, L, V))
        return np.concatenate(parts, axis=0)

    return (unshard("noisy"), unshard("adjusted"), messages, logits), res


def kernel(messages, logits, target_mask):
    out, _ = _run(messages, logits, target_mask)
    return out


# revision 44
# speedup vs baseline: 1.0129x; 1.0129x over previous
"""Trainium2 Bass kernel for nn_DeletionChannel.

Strategy
--------
Pure data parallelism: batch B=128 is sharded 16 rows per core across 8
NeuronCores. Inside a core, the 16 batch rows are laid out as 2 "halves"
of 8 rows each; the partition dim is (blk in 0..8) x (l in 0..10) = 80
partitions, and the two halves ride side by side in the free dim. All
cross-`l` mixing becomes block-diagonal constant matmuls on the tensor
engine; per-(b,l) softmax norms are per-partition scalars.

Math simplifications vs the reference:
 * The [B, 2^L, L, V] combo logsumexp collapses to a 10x10 row-stochastic
   matrix A applied in linear space: A[m,l] = sum_{c: perm[c,m]=l} exp(scl[m,c]).
 * The sequential EOS renormalization has the closed form
   col_j = ln(p_j) - ln(1 - sum_{j'<j} p_{j'}) with p the adjusted length pmf.
 * The deletion shift is a per-batch 0/1 permutation matrix built from an
   exclusive cumsum of the keep mask and an equality compare, applied as a
   bf16 matmul (all operands 0/1 -> exact); the EOS tail fill is a rank-1
   correction folded into the same PSUM accumulation group.

Performance notes (raw Bacc, manual semaphores):
 * No Tile context: Tile's kernel-tail drain/barrier machinery costs ~9us.
 * One activation-table load: the act-table pass is restricted to the
   combined natural_log_exp_and_others set so Exp/Ln never thrash the
   table SRAM (1.28us per reload otherwise).
 * The dead const-AP memsets + init all-engine barrier from Bass.__init__
   are stripped; they would otherwise start the measured window early.
 * Input and output DMAs are split across the two HWDGE rings (SP + ACT)
   so transfers and their ~2us completion latencies overlap.
"""

import numpy as np
import itertools
import math
import ml_dtypes

from concourse import bacc, bass, mybir
from concourse.bass_utils import run_bass_kernel_spmd
from concourse.mybir import ActivationFunctionType as AF, AluOpType as ALU

# The act-table insertion pass greedily picks the first table set containing
# each activation function, so alternating Exp/Ln thrashes the table SRAM
# (1.28us reload per switch). Restrict the choice to the one combined set
# (natural_log_exp_and_others: Exp+Ln+Copy+Identity) so a single load at
# kernel start covers every activation in this kernel.
_orig_get_act_tables = bacc.get_activation_tables


def _combined_act_tables(arch):
    t = _orig_get_act_tables(arch)
    return {name: (funcs if name == "natural_log_exp_and_others" else set())
            for name, funcs in t.items()}


bacc.get_activation_tables = _combined_act_tables

P_ERR = 0.1
B, L, V = 128, 10, 32
NCORES = 8
BS = B // NCORES            # batch rows per core = 16
NB = 8                      # blocks per half
NH = 2                      # halves per core
P80 = NB * L                # 80 partitions, (blk, l)
P88 = NB * (L + 1)          # 88 partitions, (blk, j)
MIN = float(np.finfo(np.float32).min)
F32 = mybir.dt.float32
BF16 = mybir.dt.bfloat16
BF = ml_dtypes.bfloat16

# bundle column layout (f32 columns; bf16 data packed 2-per-column).
# Region 1 (first DMA): inputs + everything on the critical dependency path.
HVW = V + 1                 # per-half: log(32) | mask(1)
B_HV = 0                    # [66]  h-major log/mask
B_ONE = B_HV + NH * HVW     # [1]   ones column (activation bias)
B_ZERO = B_ONE + 1          # [1]   zeros column (activation bias)
B_MSGB = B_ZERO + 1         # [32]  bf16 messages, h-major 16 cols each
B_E2 = B_MSGB + V           # [88]  block-diag identity injection 80->88
B_T = B_E2 + P88            # [88]  block-diag T (i<j), j in 0..10
B_UEXB = B_T + P88          # [40]  bf16 block-diag strict-lower cumsum
B_W0 = B_UEXB + P80 // 2    # start of the weights region (second DMA)
B_BDA = B_W0                # [80]  block-diag A^T (lhsT for expected)
B_IOTA = B_BDA + P80        # [80]  row iota 0..79
B_BLK = B_IOTA + P80        # [1]   10*blk per partition
B_ONESB = B_BLK + 1         # [40]  bf16 partition-0 ones row
B_E0B = B_ONESB + P80 // 2  # [32]  bf16 partition-0 e0-per-half row
NBUND = B_E0B + V
# const88 column layout
C_P10 = 0                   # [80]  block-diag NDLe[j, j2]
C_S10 = C_P10 + P80         # [80]  block-diag exclusive-cumsum NDLe
C_SP10 = C_S10 + P80        # [80]  P10 + S10
C_ZERO88 = C_SP10 + P80     # [1]   zeros (activation bias)
NC88 = C_ZERO88 + 1


def _host_constants():
    """A [10,10] row-stochastic mix matrix and NDLe [11,11] binomial pmf."""
    combos = np.array(list(itertools.product((0, 1), repeat=L)), dtype=bool)
    n_del = combos.sum(-1)
    combo_logits = np.log(P_ERR) * n_del + np.log1p(-P_ERR) * (L - n_del)
    not_del = np.arange(L - 1, -1, -1)[:, None] >= n_del[None, :]
    scl = np.where(not_del, combo_logits[None, :], MIN)
    m = scl.max(-1, keepdims=True)
    scl = scl - (m + np.log(np.exp(scl - m).sum(-1, keepdims=True)))  # [L, C]
    perm = np.tile(np.arange(L), (len(combos), 1))
    for i in range(1, L):
        idx = L - 1 - i
        t = combos[:, idx]
        perm[t, idx:] = np.roll(perm[t, idx:], -1, axis=1)
    A = np.zeros((L, L))
    for l in range(L):
        for lp in range(L):
            sel = scl[l, perm[:, l] == lp]
            if len(sel):
                mm = sel.max()
                if mm > MIN / 2:
                    A[l, lp] = np.exp(sel - mm).sum() * np.exp(mm)
    ndl = np.full((L + 1, L + 1), MIN)
    for n in range(L + 1):
        for k in range(n + 1):
            ndl[n, n - k] = (math.lgamma(n + 1) - math.lgamma(k + 1)
                             - math.lgamma(n - k + 1)
                             + k * math.log(P_ERR) + (n - k) * math.log(1 - P_ERR))
    NDLe = np.exp(np.where(ndl <= MIN / 2, -np.inf, ndl))
    return A, NDLe


def _pack_bf16(x):
    """Pack a [..., 2k] float array as bf16 pairs into [..., k] f32 columns."""
    xb = np.ascontiguousarray(x.astype(BF))
    assert xb.shape[-1] % 2 == 0
    return xb.view(np.uint16).view(np.uint32).view(np.float32)


def _const_blobs():
    """Constant parts of the bundle ([80, NBUND] template) and c88."""
    A, NDLe = _host_constants()
    c80 = np.zeros((P80, NBUND), np.float32)
    c88 = np.zeros((P88, NC88), np.float64)
    uex = np.zeros((P80, P80), np.float32)
    for blk in range(NB):
        r0, r1 = blk * L, (blk + 1) * L          # 80-layout rows of this block
        q0 = blk * (L + 1)                        # 88-layout base
        # BD_A[(blk,l'), (blk,l)] = A[l, l']
        c80[r0:r1, B_BDA + r0:B_BDA + r1] = A.T
        uex[r0:r1, r0:r1] = np.triu(np.ones((L, L)), k=1)
        Tm = np.zeros((L, L + 1))
        for i in range(L):
            Tm[i, i + 1:] = 1.0
        c80[r0:r1, B_T + q0:B_T + q0 + L + 1] = Tm
        E2 = np.zeros((L, L + 1))
        E2[:, :L] = np.eye(L)
        c80[r0:r1, B_E2 + q0:B_E2 + q0 + L + 1] = E2
        P10 = NDLe[:, :L]
        S10 = np.cumsum(NDLe, axis=1)[:, :L] - P10
        c88[q0:q0 + L + 1, C_P10 + r0:C_P10 + r1] = P10
        c88[q0:q0 + L + 1, C_S10 + r0:C_S10 + r1] = S10
        c88[q0:q0 + L + 1, C_SP10 + r0:C_SP10 + r1] = P10 + S10
    c80[:, B_IOTA:B_IOTA + P80] = np.arange(P80)[None, :]
    c80[:, B_BLK] = (np.arange(P80) // L) * L
    c80[:, B_ONE] = 1.0
    c80[:, B_UEXB:B_UEXB + P80 // 2] = _pack_bf16(uex)
    ones_row = np.zeros((1, P80), np.float32)
    ones_row[0, :] = 1.0
    c80[0:1, B_ONESB:B_ONESB + P80 // 2] = _pack_bf16(ones_row)
    e0 = np.zeros((1, NH * V), np.float32)
    e0[0, 0] = 1.0
    e0[0, V] = 1.0
    c80[0:1, B_E0B:B_E0B + V] = _pack_bf16(e0)
    return c80, c88.astype(np.float32)


def _strip_init_overhead(nc):
    """Remove the dead const-AP memsets and the init all-engine barrier that
    Bass.__init__ emits; nothing in this kernel reads the const APs, and all
    cross-engine ordering is established by this kernel's own semaphores."""
    b = nc.main_func.blocks[0]
    drop = [i for i in b.instructions
            if type(i).__name__ in ("InstMemset", "InstDrain",
                                    "InstEventSemaphore")]
    for i in drop:
        b.instructions.remove(i)


def build_program():
    """Raw Bacc program: manual semaphores, no Tile machinery.

    Engine streams (each internally ordered; cross-engine deps via sems):
      SP:   dma-in hv+msgb, dma-in c88, dma-out noisy, completion wait
      ACT:  dma-in weights, memzero, exp_t x2, exp_eos, log1m, p_len, logE,
            ln_p, ln_q, ln_qm, noisy copy, dma-out adjusted   (sA +1 each)
      PE:   E2, dest, T, E, P, S, SP, G0, G1, ones-fix        (sP +1 each)
      DVE:  keep, keepb, msg0-=1, rs, p_ne x2, sdest, G x2, eos_d, l1me,
            adj0, cvec, rest x2                               (sV +1 each)
    """
    nc = bacc.Bacc("TRN2", target_bir_lowering=False, debug=False)
    _strip_init_overhead(nc)
    d_bund = nc.dram_tensor("bundle", [P80, NBUND], F32, kind="ExternalInput")
    d_c88 = nc.dram_tensor("const88", [P88, NC88], F32, kind="ExternalInput")
    # outputs are partition-major [80, 2*V]; the host reassembles batch order
    d_noisy = nc.dram_tensor("noisy", [P80, NH * V], F32, kind="ExternalOutput")
    d_adj = nc.dram_tensor("adjusted", [P80, NH * V], F32, kind="ExternalOutput")

    sDb = nc.alloc_semaphore("sDb")   # hv + msgb region
    sDw = nc.alloc_semaphore("sDw")   # weights region
    sDc = nc.alloc_semaphore("sDc")   # c88
    sP = nc.alloc_semaphore("sP")
    sA = nc.alloc_semaphore("sA")
    sV = nc.alloc_semaphore("sV")
    sO = nc.alloc_semaphore("sO")

    bund = nc.alloc_sbuf_tensor("bund", [P80, NBUND], F32)
    c88 = nc.alloc_sbuf_tensor("c88", [P88, NC88], F32)
    keep = nc.alloc_sbuf_tensor("keep", [P80, NH], F32)
    keepb = nc.alloc_sbuf_tensor("keepb", [P80, NH], BF16)
    sdest = nc.alloc_sbuf_tensor("sdest", [P80, NH], F32)
    G = nc.alloc_sbuf_tensor("G", [P80, NH, P80], BF16)
    noisy_sb = nc.alloc_sbuf_tensor("noisy_sb", [P80, NH, V], F32)
    exp_t = nc.alloc_sbuf_tensor("exp_t", [P80, NH, V - 1], F32)
    s_t = nc.alloc_sbuf_tensor("s_t", [P80, NH], F32)
    rs = nc.alloc_sbuf_tensor("rs", [P80, NH], F32)
    rhsE = nc.alloc_sbuf_tensor("rhsE", [P80, NH, V - 1], F32)
    logE = nc.alloc_sbuf_tensor("logE", [P80, NH, V - 1], F32)
    exp_eos = nc.alloc_sbuf_tensor("exp_eos", [P80, NH], F32)
    log1m = nc.alloc_sbuf_tensor("log1m", [P80, NH], F32)
    p_len = nc.alloc_sbuf_tensor("p_len", [P88, NH], F32)
    ln_p = nc.alloc_sbuf_tensor("ln_p", [P80, NH], F32)
    ln_q = nc.alloc_sbuf_tensor("ln_q", [P80, NH], F32)
    ln_qm = nc.alloc_sbuf_tensor("ln_qm", [P80, NH], F32)
    l1me = nc.alloc_sbuf_tensor("l1me", [P80, NH], F32)
    adj_out = nc.alloc_sbuf_tensor("adj_out", [P80, NH, V], F32)

    dest_ps = nc.alloc_psum_tensor("dest_ps", [P80, NH], F32)
    noisy_ps = nc.alloc_psum_tensor("noisy_ps", [P80, NH, V], F32)
    E_ps = nc.alloc_psum_tensor("E_ps", [P80, NH, V - 1], F32)
    LL_ps = nc.alloc_psum_tensor("LL_ps", [P88, NH], F32)
    p_ps = nc.alloc_psum_tensor("p_ps", [P80, NH], F32)
    S_ps = nc.alloc_psum_tensor("S_ps", [P80, NH], F32)
    SP_ps = nc.alloc_psum_tensor("SP_ps", [P80, NH], F32)

    hv = bund[:, B_HV:B_HV + NH * HVW].rearrange("p (h x) -> p h x", h=NH)
    log_t = hv[:, :, 0:V]
    mask_t = hv[:, :, V:V + 1]
    msgb = bund[:, B_MSGB:B_MSGB + V].bitcast(BF16).rearrange(
        "p (h x) -> p h x", h=NH)          # [80, 2, 32] bf16
    ones80 = bund[:, B_ONE:B_ONE + 1]
    zero80 = bund[:, B_ZERO:B_ZERO + 1]
    zero88 = c88[:, C_ZERO88:C_ZERO88 + 1]
    uexb = bund[:, B_UEXB:B_UEXB + P80 // 2].bitcast(BF16)    # [80, 80]
    onesb = bund[0:1, B_ONESB:B_ONESB + P80 // 2].bitcast(BF16)  # [1, 80]
    e0b = bund[0:1, B_E0B:B_E0B + V].bitcast(BF16)            # [1, 64]

    # ---- SP: c88 input DMA ----
    nc.sync.dma_start(out=c88[:, :], in_=d_c88[:, :]).then_inc(sDc, 16)

    # ---- ACT stream (triggers both bundle DMAs on the ACT ring) ----
    # ACT (and DVE) pipelines do not interlock same-engine RAW: a consumer
    # must wait on the producer's sem tick even on the same engine.
    # The first activation (exp_eos) carries the DMA wait itself, so the act
    # table load the compiler hoists before it runs during the input DMAs,
    # and the measured window starts only when real compute starts.
    nc.scalar.dma_start(
        out=bund[:, 0:B_W0], in_=d_bund[:, 0:B_W0]).then_inc(sDb, 16)
    nc.scalar.dma_start(
        out=bund[:, B_W0:NBUND], in_=d_bund[:, B_W0:NBUND]).then_inc(sDw, 16)
    a = 0
    nc.scalar.wait_ge(sDb, 16)
    # EOS/length chain first: it feeds the longest dependency chain.
    nc.scalar.activation(exp_eos[:, :], log_t[:, :, 0], AF.Exp,
                         bias=zero80, scale=1.0).then_inc(sA, 1)
    a += 1
    nc.scalar.wait_ge(sA, a)           # same-engine RAW on exp_eos
    nc.scalar.activation(log1m[:, :], exp_eos[:, :], AF.Ln,
                         bias=ones80, scale=-1.0).then_inc(sA, 1)
    a += 1
    A_LOG1M = a
    for h in range(NH):
        nc.scalar.activation(exp_t[:, h, :], log_t[:, h, 1:V], AF.Exp,
                             bias=zero80, scale=1.0).then_inc(sA, 1)
        a += 1
    A_EXPT = a
    nc.scalar.wait_ge(sDc, 16)         # zero88 bias below lives in c88
    nc.scalar.wait_ge(sP, 3)           # LL accumulation done
    nc.scalar.activation(p_len[:, :], LL_ps[:, :], AF.Exp,
                         bias=zero88, scale=1.0).then_inc(sA, 1)
    a += 1
    A_PLEN = a
    nc.scalar.wait_ge(sP, 6)           # P, S, SP done
    nc.scalar.activation(ln_p[:, :], p_ps[:, :], AF.Ln,
                         bias=zero80, scale=1.0).then_inc(sA, 1)
    a += 1
    nc.scalar.activation(ln_q[:, :], S_ps[:, :], AF.Ln,
                         bias=ones80, scale=-1.0).then_inc(sA, 1)
    a += 1
    A_LNQ = a
    nc.scalar.activation(ln_qm[:, :], SP_ps[:, :], AF.Ln,
                         bias=ones80, scale=-1.0).then_inc(sA, 1)
    a += 1
    A_LNQM = a
    nc.scalar.wait_ge(sP, 7)           # E matmul done
    nc.scalar.activation(logE[:, :, :], E_ps[:, :, :], AF.Ln,
                         bias=zero80, scale=1.0).then_inc(sA, 1)
    a += 1
    A_LOGE = a
    nc.scalar.wait_ge(sP, 10)          # noisy matmuls done
    nc.scalar.copy(noisy_sb[:, :, :], noisy_ps[:, :, :]).then_inc(sA, 1)
    a += 1
    A_NCOPY = a

    # ---- DVE stream ----
    v = 0
    nc.vector.wait_ge(sDb, 16)
    nc.vector.tensor_scalar(
        keep[:, :], mask_t[:, :, 0], -1.0, 1.0, ALU.mult, ALU.add).then_inc(sV, 1)
    v += 1
    nc.vector.tensor_scalar(
        keepb[:, :], mask_t[:, :, 0], -1.0, 1.0, ALU.mult, ALU.add).then_inc(sV, 1)
    v += 1
    V_KEEPB = v
    # messages col0 -= 1 so the rank-1 ones fix yields the EOS tail
    nc.vector.tensor_scalar(
        msgb[:, :, 0], msgb[:, :, 0], -1.0, None, ALU.add).then_inc(sV, 1)
    v += 1
    nc.vector.wait_ge(sA, A_EXPT)
    nc.vector.tensor_reduce(
        s_t[:, :], exp_t[:, :, :], mybir.AxisListType.X,
        ALU.add).then_inc(sV, 1)
    v += 1
    nc.vector.wait_ge(sV, v)           # same-engine RAW on s_t
    nc.vector.reciprocal(rs[:, :], s_t[:, :]).then_inc(sV, 1)
    v += 1
    nc.vector.wait_ge(sV, v)           # same-engine RAW on rs
    for h in range(NH):
        nc.vector.tensor_scalar(
            rhsE[:, h, :], exp_t[:, h, :], rs[:, h:h + 1], None,
            ALU.mult).then_inc(sV, 1)
        v += 1
    V_PNE = v
    nc.vector.wait_ge(sDw, 16)         # IOTA/BLK columns live in weights rgn
    nc.vector.wait_ge(sP, 2)           # dest matmul done
    nc.vector.tensor_scalar(
        sdest[:, :], dest_ps[:, :], bund[:, B_BLK:B_BLK + 1], None,
        ALU.add).then_inc(sV, 1)
    v += 1
    nc.vector.wait_ge(sV, v)           # same-engine RAW on sdest
    for h in range(NH):
        nc.vector.tensor_scalar(
            G[:, h, :], bund[:, B_IOTA:B_IOTA + P80],
            sdest[:, h:h + 1], keep[:, h:h + 1],
            ALU.is_equal, ALU.mult).then_inc(sV, 1)
        v += 1
    V_G = v
    # adjusted col 0 = ln_p - ln_q written in place (the reference's clip to
    # <=0 only guards f32 rounding at the ~1e-7 level; p < q by construction)
    nc.vector.wait_ge(sA, A_LNQ)
    nc.vector.tensor_tensor(
        adj_out[:, :, 0], ln_p[:, :], ln_q[:, :], ALU.subtract).then_inc(sV, 1)
    v += 1
    nc.vector.wait_ge(sA, A_LNQM)
    nc.vector.tensor_tensor(
        l1me[:, :], ln_qm[:, :], ln_q[:, :], ALU.subtract).then_inc(sV, 1)
    v += 1
    V_L1ME = v
    nc.vector.wait_ge(sA, A_LOGE)
    nc.vector.wait_ge(sV, V_L1ME)      # same-engine RAW on l1me
    for h in range(NH):
        nc.vector.tensor_scalar(
            adj_out[:, h, 1:V], logE[:, h, :],
            l1me[:, h:h + 1], None, ALU.add).then_inc(sV, 1)
        v += 1
    V_ADJ = v

    # ---- PE stream ----
    p = 0
    nc.tensor.wait_ge(sDb, 16)
    nc.tensor.matmul(LL_ps[:, :], bund[:, B_E2:B_E2 + P88], log_t[:, :, 0],
                     start=True, stop=False).then_inc(sP, 1)
    p += 1
    nc.tensor.wait_ge(sV, V_KEEPB)
    nc.tensor.matmul(dest_ps[:, :], uexb, keepb[:, :]).then_inc(sP, 1)
    p += 1
    nc.tensor.wait_ge(sA, A_LOG1M)
    nc.tensor.matmul(LL_ps[:, :], bund[:, B_T:B_T + P88], log1m[:, :],
                     start=False, stop=True).then_inc(sP, 1)
    p += 1
    nc.tensor.wait_ge(sDc, 16)
    nc.tensor.wait_ge(sA, A_PLEN)
    nc.tensor.matmul(p_ps[:, :], c88[:, C_P10:C_P10 + P80],
                     p_len[:, :]).then_inc(sP, 1)
    p += 1
    nc.tensor.matmul(S_ps[:, :], c88[:, C_S10:C_S10 + P80],
                     p_len[:, :]).then_inc(sP, 1)
    p += 1
    nc.tensor.matmul(SP_ps[:, :], c88[:, C_SP10:C_SP10 + P80],
                     p_len[:, :]).then_inc(sP, 1)
    p += 1
    nc.tensor.wait_ge(sDw, 16)
    nc.tensor.wait_ge(sV, V_PNE)
    nc.tensor.matmul(E_ps.ap().rearrange("p a b -> p (a b)"),
                     bund[:, B_BDA:B_BDA + P80],
                     rhsE.ap().rearrange("p a b -> p (a b)")).then_inc(sP, 1)
    p += 1
    nc.tensor.wait_ge(sV, V_G)
    for h in range(NH):
        # start only on h=0: start zeroes the whole PSUM bank (2KB zero
        # region), so h=1 must accumulate into the already-zeroed half.
        nc.tensor.matmul(noisy_ps[:, h, :], G[:, h, :], msgb[:, h, :],
                         start=(h == 0), stop=False,
                         skip_group_check=True).then_inc(sP, 1)
        p += 1
    nc.tensor.matmul(noisy_ps.ap().rearrange("p a b -> p (a b)"),
                     onesb, e0b, start=False, stop=True,
                     skip_group_check=True).then_inc(sP, 1)
    p += 1

    # ---- output DMAs: noisy on the SP ring, adjusted on the ACT ring.
    # No final completion wait: the NEFF epilogue drains each triggering
    # engine's DMA rings before the semaphore resets. (A 4-way row-split
    # across rings was measured slower: +1.6us from the extra triggers.)
    nc.sync.wait_ge(sA, A_NCOPY)
    nc.sync.dma_start(
        out=d_noisy[:, :].rearrange("p (h v) -> p h v", h=NH),
        in_=noisy_sb[:, :, :]).then_inc(sO, 16)
    nc.scalar.wait_ge(sV, V_ADJ)
    nc.scalar.dma_start(
        out=d_adj[:, :].rearrange("p (h v) -> p h v", h=NH),
        in_=adj_out[:, :, :]).then_inc(sO, 16)

    nc.compile()
    return nc


_PROGRAM = None
_CONSTS = None


def _get_program():
    global _PROGRAM, _CONSTS
    if _PROGRAM is None:
        _PROGRAM = build_program()
        _CONSTS = _const_blobs()
    return _PROGRAM, _CONSTS


def _bundles(messages, logits, maskf, c80):
    """Per-core [80, NBUND] bundles: log/mask halves + bf16 msg + constants."""
    msg2 = messages.reshape(B * L, V)
    log2 = logits.reshape(B * L, V)
    mask2 = maskf.reshape(B * L)
    out = []
    for c in range(NCORES):
        base = c * BS * L
        bund = c80.copy()
        for h in range(NH):
            r = slice(base + h * P80, base + (h + 1) * P80)
            o = B_HV + h * HVW
            bund[:, o:o + V] = log2[r]
            bund[:, o + V] = mask2[r]
            bund[:, B_MSGB + h * (V // 2):B_MSGB + (h + 1) * (V // 2)] = (
                _pack_bf16(msg2[r]))
        out.append(bund)
    return out


def _run(messages, logits, target_mask, **spmd_kwargs):
    messages = np.ascontiguousarray(np.asarray(messages, np.float32))
    logits = np.ascontiguousarray(np.asarray(logits, np.float32))
    maskf = np.ascontiguousarray(np.asarray(target_mask).astype(np.float32))
    nc, (c80, c88) = _get_program()
    in_maps = [{"bundle": b, "const88": c88}
               for b in _bundles(messages, logits, maskf, c80)]
    res = run_bass_kernel_spmd(
        nc, in_maps, core_ids=list(range(NCORES)), **spmd_kwargs)

    def unshard(name):
        # [80, 2*V] partition-major -> batch-major [16, 10, 32] per core
        parts = []
        for c in range(NCORES):
            a = res.results[c][name].reshape(P80, NH, V)
            parts.append(np.ascontiguousarray(
                a.transpose(1, 0, 2)).reshape(BS, L, V))
        return np.concatenate(parts, axis=0)

    return (unshard("noisy"), unshard("adjusted"), messages, logits), res


def kernel(messages, logits, target_mask):
    out, _ = _run(messages, logits, target_mask)
    return out
